# revision 29
# baseline (speedup 1.0000x reference)
# Trainium2 Bass kernel for a 2-layer GraphSAGE encoder (SAGEConv mean aggr).
#
#   h   = relu(mean_nbr(x) @ W1_l + b1 + x @ W1_r)
#   out = mean_nbr(h) @ W2_l + b2 + h @ W2_r
#
# Sharding: data-parallel over destination nodes (8 cores), with the edge data
# each core needs staged to it ("halo" pre-materialized). The host permutes
# node ids (degree-balanced snake deal), pads N to 8*shard, and assigns core k
# dst rows [k*shard,(k+1)*shard). Per 128-dst batch, dst segments are packed
# into 128-slot tiles (boundaries uniform across cores so one SPMD program
# serves all 8 cores).
#
# Layer 1: messages x[src] are host-packed per core into a contiguous bf16
# slot stream [128, T, C] (pure indexing, no arithmetic) and streamed with
# large sequential DMAs -- no runtime gather. One PE matmul per tile against a
# bf16 segment matrix S1 (holding 1/deg) accumulates aggT per dst in PSUM.
# Layer 2 messages depend on h, so they are gathered at runtime: z = h @ W2_l
# is computed per core, transposed to row-major pairs [shard/2, 128] bf16
# (node 2q in cols 0:64, node 2q+1 in cols 64:128), AllGathered, and
# dma_gather'ed per slot with pair index src>>1 (fits int16, single address
# space, 256B rows) on 4 SWDGE queues. Parity-split segment matrices S2e/S2o
# select the even/odd node of each gathered pair inside the matmul; the two
# PSUM results combine as pa[0:64]+pb[64:128]. Small linear weights are
# replicated; outputs are written column-major [OC, shard] and transposed on
# the host.
import os
import sys
import numpy as np

for _p in ("/opt/trn_rl_repo",):
    if _p not in sys.path and os.path.isdir(_p):
        sys.path.append(_p)

import concourse.bass as bass
import concourse.bacc as bacc
import concourse.mybir as mybir
from concourse import tile
from concourse.bass_utils import run_bass_kernel_spmd

F32 = mybir.dt.float32
BF16 = mybir.dt.bfloat16
I16 = mybir.dt.int16
NP_BF16 = mybir.dt.np(BF16)

N_CORES = 8
BATCH = 128      # dst nodes per aggregation batch (PSUM tile width)
CT = 8           # L2 gather chunk size in 128-slot tiles (1024 idxs/call is
                 # the SWDGE ring capacity per queue with 4 queues)
CT1 = 32         # L1 stream chunk size in tiles (1MB DMAs)
NBATCH = 512     # dense-phase node batch (PSUM free-dim max for fp32)


def _cdiv(a, b):
    return -(-a // b)


# ----------------------------------------------------------------------------
# Host-side graph preprocessing (index manipulation / data staging only).
# ----------------------------------------------------------------------------
def _preprocess(x, edge_index):
    x = np.asarray(x, np.float32)
    ei = np.asarray(edge_index, np.int64)
    N, C = x.shape
    E = ei.shape[1]
    src, dst = ei[0], ei[1]

    shard = _cdiv(_cdiv(N, N_CORES), BATCH) * BATCH
    NP = shard * N_CORES
    NBT = shard // BATCH

    deg = np.bincount(dst, minlength=N).astype(np.int64)
    recip_full = (1.0 / np.maximum(deg, 1)).astype(np.float32)

    # Degree-balanced snake deal over (core, batch-of-128) bins.
    nbins = N_CORES * NBT
    order = np.argsort(-deg, kind="stable")
    i = np.arange(N)
    r = i // nbins
    p = i % nbins
    binidx = np.where(r % 2 == 0, p, nbins - 1 - p)
    core_b = binidx % N_CORES
    bat_b = binidx // N_CORES
    newid = core_b * shard + bat_b * BATCH + r
    perm = np.empty(N, np.int64)
    perm[order] = newid

    psrc = perm[src]
    pdst = perm[dst]

    x_tab = np.zeros((NP, C), np.float32)
    x_tab[perm] = x
    x_tab16 = x_tab.astype(NP_BF16)
    recip_bc = np.zeros(NP, np.float32)
    recip_bc[perm] = recip_full

    core_of = pdst // shard
    local = pdst % shard

    # per-(core, local-dst) degree
    keyd = core_of * shard + local
    degs = np.bincount(keyd, minlength=N_CORES * shard)
    degs = degs.reshape(N_CORES, shard)                 # [core, local]
    assert degs.max() <= 128, "single dst degree exceeds one tile"

    # Structural tile plan, uniform across cores: for each batch, greedily
    # split the 128 dsts into ranges where every core's segment sum fits 128
    # slots. Each dst column appears in exactly one tile => 128 S-cols/batch.
    tiles = []            # (batch, a, w) in emission order
    batch_tiles = []
    for b in range(NBT):
        d = degs[:, b * BATCH:(b + 1) * BATCH]          # [core, 128]
        csum = np.concatenate(
            [np.zeros((N_CORES, 1), np.int64), np.cumsum(d, axis=1)], axis=1)
        tl = []
        a = 0
        while a < BATCH:
            base = csum[:, a]
            w = 1
            while a + w < BATCH and ((csum[:, a + w + 1] - base) <= 128).all():
                w += 1
            tl.append((b, a, w))
            a += w
        batch_tiles.append(list(range(len(tiles), len(tiles) + len(tl))))
        tiles.extend(tl)
    T = len(tiles)

    scol_off = np.zeros(T + 1, np.int64)
    for t, (b, a, w) in enumerate(tiles):
        scol_off[t + 1] = scol_off[t] + w
    SCOLS = int(scol_off[-1])
    assert SCOLS == NBT * BATCH

    # --- per-core slot/S content -------------------------------------------
    keye = core_of * shard + local
    ordr = np.argsort(keye, kind="stable")
    psrc_s = psrc[ordr]
    keye_s = keye[ordr]
    starts = np.concatenate([[0], np.cumsum(degs.reshape(-1))])
    rank = np.arange(E) - starts[keye_s]
    core_e = keye_s // shard
    loc_e = keye_s % shard

    def wrap_idx(a_):
        return np.ascontiguousarray(
            np.tile(a_.reshape(-1, 16).T, (8, 1)).astype(np.int16))

    per_core = []
    for k in range(N_CORES):
        slot_base = np.zeros(shard, np.int64)
        S1 = np.zeros((128, SCOLS), np.float32)
        for t, (b, a, w) in enumerate(tiles):
            dloc = b * BATCH + a
            dsl = degs[k, dloc:dloc + w]
            offs = np.concatenate([[0], np.cumsum(dsl)])
            assert offs[-1] <= 128
            slot_base[dloc:dloc + w] = t * 128 + offs[:-1]
            for j in range(w):
                if dsl[j]:
                    S1[offs[j]:offs[j + 1], scol_off[t] + j] = \
                        recip_bc[k * shard + dloc + j]
        m = core_e == k
        slot = slot_base[loc_e[m]] + rank[m]
        slotsrc = np.zeros(T * 128, np.int64)
        slotsrc[slot] = psrc_s[m]

        # parity per slot, broadcast to S columns of its tile
        par_slot = (slotsrc & 1).reshape(T, 128).T      # [128, T]
        par_col = np.zeros((128, SCOLS), np.float32)
        for t in range(T):
            par_col[:, scol_off[t]:scol_off[t + 1]] = par_slot[:, t:t + 1]
        S2e = (S1 * (1.0 - par_col)).astype(NP_BF16)
        S2o = (S1 * par_col).astype(NP_BF16)

        src_grid = slotsrc.reshape(T, 128).T            # [128, T]
        msgs1 = x_tab16[src_grid].reshape(128, T * C)   # [128, T*C] bf16
        idx2 = wrap_idx(slotsrc >> 1)                   # [128, T*8] int16

        ent = {
            "msgs1": np.ascontiguousarray(msgs1),
            "idx2": idx2,
            "S1": S1.astype(NP_BF16),
            "S2e": S2e,
            "S2o": S2o,
            "xT_sh": np.ascontiguousarray(
                x_tab16.T[:, k * shard:(k + 1) * shard]),
        }
        per_core.append(ent)

    meta = dict(NP=NP, shard=shard, NBT=NBT, C=C, T=T, SCOLS=SCOLS,
                tiles=tiles, batch_tiles=batch_tiles,
                scol_off=scol_off.tolist())
    return per_core, perm, meta


# ----------------------------------------------------------------------------
# Bass program builder (one static SPMD program for all 8 cores).
# ----------------------------------------------------------------------------
def _build(meta, HID, OC):
    NP, shard, NBT, C = meta["NP"], meta["shard"], meta["NBT"], meta["C"]
    T, SCOLS = meta["T"], meta["SCOLS"]
    tiles = meta["tiles"]
    batch_tiles = meta["batch_tiles"]
    scol_off = meta["scol_off"]

    nc = bacc.Bacc("TRN2", target_bir_lowering=False, debug=False,
                   num_devices=N_CORES, num_swdge_queues=4)

    msgs1_d = nc.dram_tensor("msgs1", [128, T * C], BF16, kind="ExternalInput")
    idx2_d = nc.dram_tensor("idx2", [128, T * 8], I16, kind="ExternalInput")
    s1_d = nc.dram_tensor("S1", [128, SCOLS], BF16, kind="ExternalInput")
    s2e_d = nc.dram_tensor("S2e", [128, SCOLS], BF16, kind="ExternalInput")
    s2o_d = nc.dram_tensor("S2o", [128, SCOLS], BF16, kind="ExternalInput")
    xT_d = nc.dram_tensor("xT_sh", [C, shard], BF16, kind="ExternalInput")
    ident_d = nc.dram_tensor("ident", [128, 128], BF16, kind="ExternalInput")
    w1l_d = nc.dram_tensor("W1_l", [C, HID], BF16, kind="ExternalInput")
    w1r_d = nc.dram_tensor("W1_r", [C, HID], BF16, kind="ExternalInput")
    w2l_d = nc.dram_tensor("W2_l", [HID, OC], BF16, kind="ExternalInput")
    w2r_d = nc.dram_tensor("W2_r", [HID, OC], BF16, kind="ExternalInput")
    b1_d = nc.dram_tensor("b1", [HID, 1], F32, kind="ExternalInput")
    b2_d = nc.dram_tensor("b2", [OC, 1], F32, kind="ExternalInput")
    out_d = nc.dram_tensor("out", [OC, shard], F32, kind="ExternalOutput")

    NB = _cdiv(shard, NBATCH)
    NC1 = _cdiv(T, CT1)
    NC2 = _cdiv(T, CT)

    with tile.TileContext(nc) as tc:
        with (
            tc.tile_pool(name="res", bufs=1) as rp,
            tc.tile_pool(name="m1p", bufs=4) as m1p,
            tc.tile_pool(name="m2p", bufs=6) as m2p,
            tc.tile_pool(name="stage", bufs=3) as stp,
            tc.tile_pool(name="pA", bufs=2, space="PSUM") as pA,
            tc.tile_pool(name="pB", bufs=2, space="PSUM") as pB,
            tc.tile_pool(name="pD", bufs=2, space="PSUM") as pD,
            tc.tile_pool(name="pT", bufs=2, space="PSUM") as pT,
            tc.tile_pool(name="dram", bufs=1, space="DRAM") as dram_p,
        ):
            def load(shape, dtype, dram_t, name):
                # One-shot loads of persistent tiles go on the ACT HWDGE ring:
                # no buffer-reuse waits (fresh tiles), and it keeps them off
                # the sync ring, which carries the msgs1 stream + ag_in/out
                # writes whose drain order gates the AllGather start.
                t = rp.tile(shape, dtype, name=name, tag=name)
                nc.scalar.dma_start(t[:], dram_t.ap())
                return t

            # L1-critical loads first; L2-only tensors load after the msgs
            # stream below so they don't delay it on the in-order sync queue.
            s1_sb = load([128, SCOLS], BF16, s1_d, "s1_sb")
            xT_sb = load([C, shard], BF16, xT_d, "xT_sb")
            w1l_sb = load([C, HID], BF16, w1l_d, "w1l_sb")
            w1r_sb = load([C, HID], BF16, w1r_d, "w1r_sb")
            w2l_sb = load([HID, OC], BF16, w2l_d, "w2l_sb")
            w2r_sb = load([HID, OC], BF16, w2r_d, "w2r_sb")
            b1_sb = load([HID, 1], F32, b1_d, "b1_sb")
            b2_sb = load([OC, 1], F32, b2_d, "b2_sb")
            ident_sb = load([128, 128], BF16, ident_d, "ident_sb")

            agg_sb = rp.tile([C, shard], BF16, name="agg_sb", tag="agg_sb")
            hT_sb = rp.tile([HID, shard], BF16, name="hT_sb", tag="hT_sb")
            zT_sb = rp.tile([OC, shard], BF16, name="zT_sb", tag="zT_sb")
            aggz_sb = rp.tile([OC, shard], F32, name="aggz_sb", tag="aggz_sb")
            root_sb = rp.tile([OC, shard], F32, name="root_sb", tag="root_sb")

            ag_in = dram_p.tile([shard // 2, 128], BF16, name="ag_in")
            z_full = dram_p.tile([NP // 2, 128], BF16, name="z_full",
                                 addr_space="Shared")

            # ---- layer 1: stream host-packed messages, aggregate ----
            chunks1 = {}
            for ci in range(NC1):
                c0 = ci * CT1
                nt = min(CT1, T - c0)
                m1 = m1p.tile([128, CT1 * C], BF16, name="m1", tag="m1")
                nc.sync.dma_start(m1[:, :nt * C],
                                  msgs1_d.ap()[:, c0 * C:(c0 + nt) * C])
                chunks1[ci] = m1

            idx2_sb = load([128, T * 8], I16, idx2_d, "idx2_sb")
            s2e_sb = load([128, SCOLS], BF16, s2e_d, "s2e_sb")
            s2o_sb = load([128, SCOLS], BF16, s2o_d, "s2o_sb")

            for b in range(NBT):
                psum = pA.tile([128, BATCH], F32, name="psum1", tag="pa")
                for t in batch_tiles[b]:
                    _, a, w = tiles[t]
                    mt = chunks1[t // CT1][:, (t % CT1) * C:(t % CT1 + 1) * C]
                    nc.tensor.matmul(
                        psum[:, a:a + w], mt,
                        s1_sb[:, scol_off[t]:scol_off[t] + w],
                        start=True, stop=True)
                nc.scalar.activation(
                    agg_sb[:, b * BATCH:(b + 1) * BATCH], psum[:],
                    mybir.ActivationFunctionType.Copy)

            # ---- layer 1 dense + z + pair-transpose ----
            for nb in range(NB):
                w = min(NBATCH, shard - nb * NBATCH)
                blk = slice(nb * NBATCH, nb * NBATCH + w)
                dp = pD.tile([128, NBATCH], F32, name="dp", tag="dp")
                nc.tensor.matmul(dp[:HID, :w], w1l_sb[:], agg_sb[:, blk],
                                 start=True, stop=False)
                nc.tensor.matmul(dp[:HID, :w], w1r_sb[:], xT_sb[:, blk],
                                 start=False, stop=True)
                nc.scalar.activation(
                    hT_sb[:, blk], dp[:HID, :w],
                    mybir.ActivationFunctionType.Relu, bias=b1_sb[:])
                zp = pD.tile([128, NBATCH], F32, name="zp", tag="dp")
                nc.tensor.matmul(zp[:OC, :w], w2l_sb[:], hT_sb[:, blk],
                                 start=True, stop=True)
                nc.vector.tensor_copy(zT_sb[:, blk], zp[:OC, :w])
                for q0 in range(nb * NBATCH, nb * NBATCH + w, 128):
                    tp = pT.tile([OC, 128], BF16, name="tp", tag="tp")
                    nc.tensor.transpose(tp[:, 0:64], zT_sb[:, q0:q0 + 128:2],
                                        ident_sb[:OC, :OC])
                    nc.tensor.transpose(tp[:, 64:128],
                                        zT_sb[:, q0 + 1:q0 + 128:2],
                                        ident_sb[:OC, :OC])
                    zs = stp.tile([OC, 128], BF16, name="zs", tag="zs")
                    nc.vector.tensor_copy(zs[:], tp[:])
                    nc.sync.dma_start(ag_in[q0 // 2:q0 // 2 + 64, :], zs[:])

            # L2 root term (with bias folded in) computed here: tensor/ACT are
            # otherwise idle while the AllGather runs, and this removes the
            # 13 root matmuls + activations from the post-gather tail.
            for nb in range(NB):
                w = min(NBATCH, shard - nb * NBATCH)
                blk = slice(nb * NBATCH, nb * NBATCH + w)
                rt = pD.tile([128, NBATCH], F32, name="rt", tag="dp")
                nc.tensor.matmul(rt[:OC, :w], w2r_sb[:], hT_sb[:, blk],
                                 start=True, stop=True)
                nc.scalar.activation(
                    root_sb[:, blk], rt[:OC, :w],
                    mybir.ActivationFunctionType.Identity, bias=b2_sb[:])

            nc.gpsimd.collective_compute(
                "AllGather", mybir.AluOpType.bypass,
                replica_groups=[list(range(N_CORES))],
                ins=[ag_in.opt()], outs=[z_full.opt()])

            # ---- layer 2: runtime gather of z pairs, parity aggregation ----
            chunks2 = {}
            for ci in range(NC2):
                c0 = ci * CT
                nt = min(CT, T - c0)
                m2 = m2p.tile([128, CT, C], BF16, name="m2", tag="m2")
                nc.gpsimd.dma_gather(
                    out_ap=m2[:, :nt, :],
                    in_ap=z_full[:],
                    idxs_ap=idx2_sb[:, c0 * 8:(c0 + nt) * 8],
                    num_idxs=nt * 128,
                    num_idxs_reg=nt * 128,
                    elem_size=C,
                    queue_num=ci % 4,
                )
                chunks2[ci] = m2

            for b in range(NBT):
                pa = pA.tile([128, BATCH], F32, name="pa", tag="pa")
                pb = pB.tile([128, BATCH], F32, name="pb", tag="pb")
                for t in batch_tiles[b]:
                    _, a, w = tiles[t]
                    mt = chunks2[t // CT][:, t % CT, :]
                    nc.tensor.matmul(
                        pa[:, a:a + w], mt,
                        s2e_sb[:, scol_off[t]:scol_off[t] + w],
                        start=True, stop=True)
                    nc.tensor.matmul(
                        pb[:, a:a + w], mt,
                        s2o_sb[:, scol_off[t]:scol_off[t] + w],
                        start=True, stop=True)
                pb_sb = stp.tile([OC, BATCH], F32, name="pb_sb", tag="pb_sb")
                nc.scalar.activation(pb_sb[:], pb[OC:2 * OC, :],
                                     mybir.ActivationFunctionType.Copy)
                nc.vector.tensor_tensor(
                    aggz_sb[:, b * BATCH:(b + 1) * BATCH],
                    pa[0:OC, :], pb_sb[:], mybir.AluOpType.add)

            # ---- layer 2 output: precomputed root (+bias) plus aggz ----
            for nb in range(NB):
                w = min(NBATCH, shard - nb * NBATCH)
                blk = slice(nb * NBATCH, nb * NBATCH + w)
                o2 = stp.tile([OC, NBATCH], F32, name="o2", tag="o2")
                nc.vector.tensor_tensor(o2[:, :w], root_sb[:, blk],
                                        aggz_sb[:, blk], mybir.AluOpType.add)
                nc.sync.dma_start(out_d.ap()[:, blk], o2[:, :w])

    nc.compile()
    return nc


_CACHE = {}


def _prepare(x, edge_index, W1_l, b1, W1_r, W2_l, b2, W2_r):
    x = np.asarray(x, np.float32)
    W1_l = np.asarray(W1_l, np.float32)
    W1_r = np.asarray(W1_r, np.float32)
    W2_l = np.asarray(W2_l, np.float32)
    W2_r = np.asarray(W2_r, np.float32)
    b1 = np.asarray(b1, np.float32)
    b2 = np.asarray(b2, np.float32)
    HID = W1_l.shape[1]
    OC = W2_l.shape[1]
    N = x.shape[0]

    per_core, perm, meta = _preprocess(x, edge_index)

    key = (meta["NP"], meta["T"], meta["SCOLS"],
           tuple(meta["tiles"]), HID, OC)
    if key not in _CACHE:
        _CACHE[key] = _build(meta, HID, OC)
    nc = _CACHE[key]

    ident = np.eye(128, dtype=np.float32).astype(NP_BF16)
    shared = {
        "ident": ident,
        "W1_l": W1_l.astype(NP_BF16), "W1_r": W1_r.astype(NP_BF16),
        "W2_l": W2_l.astype(NP_BF16), "W2_r": W2_r.astype(NP_BF16),
        "b1": b1.reshape(HID, 1).copy(), "b2": b2.reshape(OC, 1).copy(),
    }
    in_maps = []
    for k in range(N_CORES):
        m = dict(shared)
        m.update(per_core[k])
        in_maps.append(m)
    return nc, in_maps, perm, N


def kernel(x, edge_index, W1_l, b1, W1_r, W2_l, b2, W2_r):
    nc, in_maps, perm, N = _prepare(x, edge_index, W1_l, b1, W1_r,
                                    W2_l, b2, W2_r)
    res = run_bass_kernel_spmd(nc, in_maps, core_ids=list(range(N_CORES)))
    out_full = np.concatenate(
        [res.results[k]["out"] for k in range(N_CORES)], axis=1)
    return np.ascontiguousarray(out_full.T[perm[:N]].astype(np.float32))
